# revision 29
# baseline (speedup 1.0000x reference)
"""Entmax-alpha Bass kernel for Trainium2, 8-core SPMD — v3.

Algorithm (vs v2's AB regula falsi with 5 pow-evals / 10 big ACT passes per
supertile at 606us): fitted initializer + safeguarded Newton + Taylor-
corrected output eval.

  1. init (1 ACT pass): Z = sum exp(4*(x-mx)). The root's temperature
     c* = s*(mx-tau*) is tightly predicted per row by a per-head quadratic
     in lnZ (coefficients fitted offline on synthetic N(0,1) rows,
     interpolated in s from FIT_TABLE). t0 = mx - c/s, clipped to the
     guaranteed bracket [mx-1/s, mx-K^{-s}/s].
  2. 1 (easy head) or 2 (hard head, s>=0.55) Newton evals, 2 ACT passes
     each: u = max(x-t, eps); L = ln(s*u); w1 = exp((p-1)*L).  Then
     S = s*sum(w1*u) (exact: w1*u = (su)^p / s) and S1 = sum(w1) -
     n_clamped*(s*eps)^(p-1) (the clamp plateau is removed exactly via a
     compare+accum count).  Newton in h=lnS space: dt = s*h*S/S1 (p*s==1).
     f is convex decreasing so Newton never overshoots from the f>=1 side;
     steps are clipped to the static bracket.
  3. output eval (2 ACT passes + Exp1): L, w = exp(p*L) (accum -> S),
     w1 = exp((p-1)*L); the *next* Newton step is applied as a first-order
     elementwise Taylor update wc = w - (h*S/S1')*w1*s... with p*s==1 the
     correction scalar is just h*S/S1; wc is renormalized by its exact sum
     (accum) and clamped at 0 inside the final normalize (op1=max).
     Equivalent accuracy to a full extra eval at ~1/3 the ACT cost.

  I/O: host uploads x pre-cast to f16 (halves DMA) and reads f16 y back.
  x streams from HBM per eval (no SBUF residency; DMA has large slack).
  All row sums ride scalar_tensor_tensor accum_out in 2x/4x DVE modes or
  ACT accum.  numpy study worst-row |y err|: <=2.7e-3 easy, <=2.1e-3 hard
  (harness gate 2e-2).
"""

import os as _os

import numpy as np

import concourse.bacc as bacc
import concourse.mybir as mybir
from concourse.tile import TileContext
from concourse.bass_utils import run_bass_kernel_spmd

B, H, Q, K = 4, 16, 1024, 1024
NCORES = 8
BLOCKS = (B * H) // NCORES      # head-blocks per core (8)
R = 4                           # q-subrows per partition per supertile
ST_ROWS = 128 * R               # rows per supertile (512)
N_ST = BLOCKS * Q // ST_ROWS    # supertiles per core (16)
NC = N_ST * R                   # state columns (64)
KH = K // 2

BETA = 4.0                      # init Exp scale (matches FIT_TABLE fit)
UEPS = float(2.0 ** -23)        # clamp floor for the Ln input (exact f16)
HARD_S = float(_os.environ.get("HARDS", "0.55"))
XHARD_S = float(_os.environ.get("XHARDS", "0.70"))
LOOKAHEAD = int(_os.environ.get("LOOKAHEAD", "3"))
DELAY = int(_os.environ.get("DELAY", "2"))
OBGAP = int(_os.environ.get("OBGAP", "3"))
FBURST = int(_os.environ.get("FBURST", "8"))
ACC_INIT = _os.environ.get("ACCINIT", "0") == "1"   # init Z via ACT accum
ACC_S = _os.environ.get("ACCS", "0") == "1"         # OA S via ACT accum
CPOOL = int(_os.environ.get("CPOOL", "2"))          # clamp subrows on Pool
SPOOL = int(_os.environ.get("SPOOL", "0"))          # fold subrows on Pool
OBPOOL = int(_os.environ.get("OBPOOL", "0"))        # corr subrows on Pool
PSUM_L = _os.environ.get("PSUML", "1") == "1"

# lc ~ a + b*lnZ + c*lnZ^2 fitted on synthetic N(0,1) rows, K=1024, beta=4
FIT_TABLE = [
    (0.0400, -0.1163303, -0.0414107, +0.0076434),
    (0.0800, -0.1912820, -0.0863653, +0.0147054),
    (0.1200, -0.2329487, -0.1340005, +0.0218060),
    (0.1600, -0.2495165, -0.1863370, +0.0304684),
    (0.2000, -0.2572720, -0.2289561, +0.0352949),
    (0.2400, -0.2556715, -0.2690348, +0.0390864),
    (0.2800, -0.2496982, -0.3072143, +0.0430376),
    (0.3200, -0.2448862, -0.3364092, +0.0437538),
    (0.3600, -0.2305074, -0.3786700, +0.0503131),
    (0.4000, -0.2158057, -0.4129058, +0.0530508),
    (0.4400, -0.2088186, -0.4307900, +0.0499715),
    (0.4800, -0.1916382, -0.4675049, +0.0551991),
    (0.5200, -0.1826100, -0.4878948, +0.0542270),
    (0.5600, -0.1674966, -0.5179781, +0.0586533),
    (0.6000, -0.1577472, -0.5331484, +0.0558217),
    (0.6400, -0.1447803, -0.5551327, +0.0563687),
    (0.6800, -0.1283865, -0.5840495, +0.0592415),
    (0.7200, -0.1095254, -0.6217898, +0.0690736),
    (0.7600, -0.1087213, -0.6163985, +0.0585828),
    (0.8000, -0.0942934, -0.6405565, +0.0602640),
    (0.8400, -0.0818819, -0.6659166, +0.0671857),
    (0.8800, -0.0788647, -0.6583176, +0.0556963),
    (0.9200, -0.0652865, -0.6792696, +0.0572530),
    (0.9600, -0.0577361, -0.6821607, +0.0496152),
    (1.0000, -0.0342309, -0.7223812, +0.0618745),
]

AL = mybir.AluOpType
AF = mybir.ActivationFunctionType
F32 = mybir.dt.float32
F16 = mybir.dt.float16

LAST_RESULT = None


def _fit_coeffs(s):
    xs = np.array([r[0] for r in FIT_TABLE])
    return [float(np.interp(s, xs, np.array([r[ci] for r in FIT_TABLE])))
            for ci in range(1, 4)]


def _build(n_x_st, n_hard_st):
    nc = bacc.Bacc(None, target_bir_lowering=False)
    x_in = nc.declare_dram_parameter("x", [BLOCKS * Q, K], F16,
                                     isOutput=False)
    cst_in = nc.declare_dram_parameter("cst", [128, 10 * NC], F32,
                                       isOutput=False)
    y_out = nc.declare_dram_parameter("y", [BLOCKS * Q, K], F16,
                                      isOutput=True)

    with TileContext(nc) as tc:
        with tc.tile_pool(name="state", bufs=1) as stp, \
             tc.tile_pool(name="xt", bufs=4) as xp, \
             tc.tile_pool(name="work", bufs=3) as wp, \
             tc.tile_pool(name="keep", bufs=3) as kp, \
             tc.tile_pool(name="scr", bufs=2) as scp, \
             (tc.tile_pool(name="lps", bufs=1, space="PSUM") if PSUM_L
              else tc.tile_pool(name="lsb", bufs=1)) as lp:
            v = nc.vector
            g = nc.gpsimd

            cst = stp.tile([128, 10 * NC], F32)
            nc.sync.dma_start(cst[:, :], cst_in[:, :])
            sC = cst[:, 0 * NC:1 * NC]     # s
            pC = cst[:, 1 * NC:2 * NC]     # p = 1/s
            pm1C = cst[:, 2 * NC:3 * NC]   # p - 1
            isC = cst[:, 3 * NC:4 * NC]    # 1/s
            kisC = cst[:, 4 * NC:5 * NC]   # K^{-s}/s
            aC = cst[:, 5 * NC:6 * NC]     # init fit a
            bC = cst[:, 6 * NC:7 * NC]     # init fit b
            cC = cst[:, 7 * NC:8 * NC]     # init fit c2
            lminC = cst[:, 8 * NC:9 * NC]  # -s*ln(K)
            epwC = cst[:, 9 * NC:10 * NC]  # (s*UEPS)^(p-1)

            mx = stp.tile([128, NC], F32)
            tS = stp.tile([128, NC], F32)
            tpS = stp.tile([128, NC], F32)
            tnS = stp.tile([128, NC], F32)
            SS = stp.tile([128, NC], F32)   # S accum slots
            S1 = stp.tile([128, NC], F32)   # sum(w1) accum slots
            SxS = stp.tile([128, NC], F32)  # sum(w1*u) accum slots
            cnt = stp.tile([128, NC], F32)  # clamped-element counts
            naS = stp.tile([128, NC], F32)  # -(h*S/S1) for the correction
            t1 = stp.tile([128, NC], F32)
            t2 = stp.tile([128, NC], F32)
            t3 = stp.tile([128, NC], F32)

            # touch ACT so the one Ln/Exp table load overlaps the first DMA
            v.memset(t1[:, 0:1], 1.0)
            nc.scalar.activation(t1[:, 0:1], t1[:, 0:1], AF.Ln)

            def x_dram_ap(handle, st):
                r0 = st * ST_ROWS
                return handle[r0:r0 + ST_ROWS, :].rearrange(
                    "(j p) k -> p j k", p=128)

            def sb3(tile_ap):
                return tile_ap.rearrange("p (j k) -> p j k", k=K)

            # ---------- item stream ----------
            # xhard STs: L,E,E,E,OA,OB; hard: L,E,E,OA,OB; easy: L,E,OA,OB
            n_hc = n_x_st + n_hard_st
            def interleave(pairs):
                # pairs: list of (first, second) two-phase items per st;
                # emit firsts leading seconds by OBGAP
                out = []
                firsts = [p[0] for p in pairs]
                seconds = [p[1] for p in pairs]
                n = len(pairs)
                fi = si = 0
                while si < n:
                    if fi < n and fi - si < OBGAP:
                        out.append(firsts[fi])
                        fi += 1
                    else:
                        out.append(seconds[si])
                        si += 1
                return out

            xhard = list(range(n_x_st))
            hard = list(range(n_x_st, n_hc))
            easy = list(range(n_hc, N_ST))
            items = []
            items += [("L", st) for st in range(N_ST)]
            items += [("E", st) for st in range(N_ST)]
            items += [("E", st) for st in xhard + hard]
            items += [("E", st) for st in xhard]
            items += interleave([(("OA", st), ("OB", st)) for st in easy])
            items += interleave([(("OA", st), ("OB", st))
                                 for st in xhard + hard])
            n_items = len(items)
            ob_of = {}
            for i, (kk, st) in enumerate(items):
                if kk == "OB":
                    ob_of[st] = i

            live = {}

            def clamp_feed(st):
                """DMA x, clamp u = max(x-t, eps)."""
                cc = st * R
                xt = xp.tile([128, R * K], F16, name="xt")
                nc.sync.dma_start(sb3(xt[:, :]), x_dram_ap(x_in, st))
                u16 = wp.tile([128, R * K], F16, name="u16")
                for j in range(R):
                    sl = slice(j * K, (j + 1) * K)
                    eng = g if j < CPOOL else v
                    eng.tensor_scalar(u16[:, sl], xt[:, sl],
                                      tS[:, cc + j:cc + j + 1], UEPS,
                                      op0=AL.subtract, op1=AL.max)
                return (u16,)

            def pre(idx):
                kind, st = items[idx]
                cc = st * R
                if kind == "L":
                    xt = xp.tile([128, R * K], F16, name="xt")
                    nc.sync.dma_start(sb3(xt[:, :]), x_dram_ap(x_in, st))
                    m1 = scp.tile([128, R * KH], F16, name="m1")
                    a3 = xt[:, :].rearrange("p (j two k) -> p j two k",
                                            two=2, k=KH)
                    v.tensor_tensor(
                        m1[:, :].rearrange("p (j k) -> p j k", k=KH),
                        a3[:, :, 0, :], a3[:, :, 1, :], op=AL.max)
                    m2 = scp.tile([128, R * (KH // 2)], F16, name="m2")
                    b3 = m1[:, :].rearrange("p (j two k) -> p j two k",
                                            two=2, k=KH // 2)
                    v.tensor_tensor(
                        m2[:, :].rearrange("p (j k) -> p j k", k=KH // 2),
                        b3[:, :, 0, :], b3[:, :, 1, :], op=AL.max)
                    v.tensor_reduce(
                        mx[:, cc:cc + R],
                        m2[:, :].rearrange("p (j k) -> p j k", k=KH // 2),
                        axis=mybir.AxisListType.X, op=AL.max)
                    d16 = wp.tile([128, R * K], F16, name="u16")
                    for j in range(R):
                        sl = slice(j * K, (j + 1) * K)
                        eng = g if j < CPOOL else v
                        eng.tensor_scalar(d16[:, sl], xt[:, sl],
                                          mx[:, cc + j:cc + j + 1], None,
                                          op0=AL.subtract)
                    live[idx] = (d16,)
                elif kind in ("E", "OA"):
                    live[idx] = clamp_feed(st)

            def fold_sum(src, dstS, cc):
                zf = scp.tile([128, R * KH], F16, name="zf")
                for j in range(R):
                    eng = g if j < SPOOL else v
                    eng.scalar_tensor_tensor(
                        zf[:, j * KH:(j + 1) * KH],
                        src[:, j * K:j * K + KH], 0.0,
                        src[:, j * K + KH:(j + 1) * K],
                        op0=AL.add, op1=AL.add,
                        accum_out=dstS[:, cc + j:cc + j + 1])

            def post(idx):
                kind, st = items[idx]
                cc = st * R
                if kind == "L":
                    (d16,) = live.pop(idx)
                    if ACC_INIT:
                        e16 = scp.tile([128, K], F16, name="e16")
                        for j in range(R):
                            nc.scalar.activation(
                                e16[:, :], d16[:, j * K:(j + 1) * K],
                                AF.Exp, scale=BETA,
                                accum_out=SS[:, cc + j:cc + j + 1])
                    else:
                        e16 = wp.tile([128, R * K], F16, name="w1s")
                        nc.scalar.activation(e16[:, :], d16[:, :], AF.Exp,
                                             scale=BETA)
                        fold_sum(e16, SS, cc)
                    return
                if kind == "OB":
                    w16, w1 = live.pop(idx)
                    wc = wp.tile([128, R * K], F16, name="u16")
                    for j in range(R):
                        sl = slice(j * K, (j + 1) * K)
                        eng = g if j < OBPOOL else v
                        eng.scalar_tensor_tensor(
                            wc[:, sl], w1[:, sl],
                            naS[:, cc + j:cc + j + 1], w16[:, sl],
                            op0=AL.mult, op1=AL.add,
                            accum_out=SS[:, cc + j:cc + j + 1])
                    v.reciprocal(t1[:, cc:cc + R], SS[:, cc:cc + R])
                    dap = x_dram_ap(y_out, st)
                    for j in range(R):
                        sl = slice(j * K, (j + 1) * K)
                        eng = g if j < CPOOL else v
                        eng.tensor_scalar(wc[:, sl], wc[:, sl],
                                          t1[:, cc + j:cc + j + 1], 0.0,
                                          op0=AL.mult, op1=AL.max)
                    nc.sync.dma_start(dap[:, 0:2, :], sb3(wc[:, 0:2 * K]))
                    nc.sync.dma_start(dap[:, 2:4, :],
                                      sb3(wc[:, 2 * K:4 * K])[:, 0:2, :])
                    return
                (u16,) = live.pop(idx)
                Lt = lp.tile([128, R * K], F32, name="L", tag="L")
                nc.scalar.activation(Lt[:, :], u16[:, :], AF.Ln,
                                     scale=sC[:, cc:cc + 1])
                if kind == "E":
                    # w1 = exp((p-1)L) with accum -> S1; S = s*sum(w1*u)
                    w1 = wp.tile([128, R * K], F16, name="w1s")
                    for j in range(R):
                        sl = slice(j * K, (j + 1) * K)
                        nc.scalar.activation(
                            w1[:, sl], Lt[:, sl], AF.Exp,
                            scale=pm1C[:, cc:cc + 1],
                            accum_out=S1[:, cc + j:cc + j + 1])
                    xw = wp.tile([128, R * K], F16, name="xw")
                    v.tensor_tensor(xw[:, :], u16[:, :], w1[:, :],
                                    op=AL.mult)
                    fold_sum(xw, SS, cc)
                else:  # OA: both exps with accum; keep w, w1 for OB
                    w16 = kp.tile([128, R * K], F16, name="w16")
                    for j in range(R):
                        sl = slice(j * K, (j + 1) * K)
                        nc.scalar.activation(
                            w16[:, sl], Lt[:, sl], AF.Exp,
                            scale=pC[:, cc:cc + 1],
                            accum_out=SS[:, cc + j:cc + j + 1])
                    w1 = kp.tile([128, R * K], F16, name="w1k")
                    if ACC_S:
                        for j in range(R):
                            sl = slice(j * K, (j + 1) * K)
                            nc.scalar.activation(
                                w1[:, sl], Lt[:, sl], AF.Exp,
                                scale=pm1C[:, cc:cc + 1],
                                accum_out=S1[:, cc + j:cc + j + 1])
                    else:
                        nc.scalar.activation(w1[:, :], Lt[:, :], AF.Exp,
                                             scale=pm1C[:, cc:cc + 1])
                        fold_sum(w1, S1, cc)
                    live[ob_of[st]] = (w16, w1)
                if st < n_hc:
                    # (s*eps)^(p-1) plateau pollutes S1 when p-1 < 1:
                    # count clamped elements, subtract exactly in update()
                    mk = scp.tile([128, R * K], F16, name="mk")
                    v.tensor_scalar(mk[:, :], u16[:, :], UEPS * 1.5, None,
                                    op0=AL.is_le)
                    fold_sum(mk, cnt, cc)

            def update_run(kind, st0, st1):
                st = st0
                cg = slice(st0 * R, st1 * R + R)
                if kind == "OB":
                    return
                if kind == "L":
                    # t0 = mx - exp(clip(a + lnZ*(b + c*lnZ), lmin, 0))/s
                    nc.scalar.activation(t1[:, cg], SS[:, cg], AF.Ln)
                    v.tensor_tensor(t2[:, cg], cC[:, cg], t1[:, cg],
                                    op=AL.mult)
                    v.tensor_tensor(t2[:, cg], t2[:, cg], bC[:, cg],
                                    op=AL.add)
                    v.tensor_tensor(t2[:, cg], t2[:, cg], t1[:, cg],
                                    op=AL.mult)
                    v.tensor_tensor(t2[:, cg], t2[:, cg], aC[:, cg],
                                    op=AL.add)
                    v.tensor_tensor(t2[:, cg], t2[:, cg], lminC[:, cg],
                                    op=AL.max)
                    v.tensor_scalar_min(t2[:, cg], t2[:, cg], 0.0)
                    nc.scalar.activation(t2[:, cg], t2[:, cg], AF.Exp)
                    v.tensor_tensor(t2[:, cg], t2[:, cg], isC[:, cg],
                                    op=AL.mult)
                    v.tensor_tensor(tS[:, cg], mx[:, cg], t2[:, cg],
                                    op=AL.subtract)
                    v.tensor_tensor(tpS[:, cg], mx[:, cg], isC[:, cg],
                                    op=AL.subtract)
                    v.tensor_tensor(tnS[:, cg], mx[:, cg], kisC[:, cg],
                                    op=AL.subtract)
                    return
                # E / OA: h = ln S; q = h*S/S1  (S1 = sum (su)^(p-1))
                if st < n_hc:
                    # exact removal of the eps-plateau from S1
                    v.tensor_tensor(t2[:, cg], cnt[:, cg], epwC[:, cg],
                                    op=AL.mult)
                    v.tensor_tensor(t2[:, cg], S1[:, cg], t2[:, cg],
                                    op=AL.subtract)
                    v.tensor_scalar_max(t2[:, cg], t2[:, cg], 1e-30)
                else:
                    v.tensor_scalar_max(t2[:, cg], S1[:, cg], 1e-30)
                v.reciprocal(t2[:, cg], t2[:, cg])
                if kind == "E":
                    # SS holds sum(w1*u); S = s*SS.  dt = h*S/S1.
                    v.tensor_tensor(t3[:, cg], SS[:, cg], sC[:, cg],
                                    op=AL.mult)
                    nc.scalar.activation(t1[:, cg], t3[:, cg], AF.Ln)
                    v.tensor_tensor(t1[:, cg], t1[:, cg], SS[:, cg],
                                    op=AL.mult)
                    v.tensor_tensor(t1[:, cg], t1[:, cg], t2[:, cg],
                                    op=AL.mult)
                    v.tensor_tensor(t1[:, cg], t1[:, cg], sC[:, cg],
                                    op=AL.mult)
                    v.tensor_tensor(tS[:, cg], tS[:, cg], t1[:, cg],
                                    op=AL.add)
                    v.tensor_tensor(tS[:, cg], tS[:, cg], tpS[:, cg],
                                    op=AL.max)
                    v.tensor_tensor(tS[:, cg], tS[:, cg], tnS[:, cg],
                                    op=AL.min)
                else:  # OA: SS holds S.  dhat = h*S/S1, bracket-clipped
                    nc.scalar.activation(t1[:, cg], SS[:, cg], AF.Ln)
                    v.tensor_tensor(t1[:, cg], t1[:, cg], SS[:, cg],
                                    op=AL.mult)
                    v.tensor_tensor(t1[:, cg], t1[:, cg], t2[:, cg],
                                    op=AL.mult)
                    # clip dhat into [tp - t, tn - t] (guards S1 blowups)
                    v.tensor_tensor(t2[:, cg], tpS[:, cg], tS[:, cg],
                                    op=AL.subtract)
                    v.tensor_tensor(t1[:, cg], t1[:, cg], t2[:, cg],
                                    op=AL.max)
                    v.tensor_tensor(t2[:, cg], tnS[:, cg], tS[:, cg],
                                    op=AL.subtract)
                    v.tensor_tensor(t1[:, cg], t1[:, cg], t2[:, cg],
                                    op=AL.min)
                    v.tensor_scalar_mul(naS[:, cg], t1[:, cg], -1.0)

            # ---------- pipelined emission ----------
            # updates flush in bursts so contiguous same-kind runs batch
            # into single [128, n*R] ops (saves tiny-op overhead); any
            # pre/post that depends on an st's pending update force-flushes
            # it first.
            def flush(entries):
                runs = []
                for (due, idx) in entries:
                    kk, st = items[idx]
                    if (runs and runs[-1][0] == kk
                            and st == runs[-1][2] + 1 and kk != "OB"
                            and (st < n_hc) == (runs[-1][1] < n_hc)):
                        runs[-1][2] = st
                    else:
                        runs.append([kk, st, st])
                for kk, st0, st1 in runs:
                    update_run(kk, st0, st1)

            pending = []

            def flush_due(i, need_st=None):
                take = [e for e in pending
                        if e[0] <= i
                        or (need_st is not None and items[e[1]][1] == need_st)]
                if not take:
                    return
                for e in take:
                    pending.remove(e)
                flush(sorted(take, key=lambda e: e[1]))

            for i in range(min(LOOKAHEAD, n_items)):
                pre(i)
            for i in range(n_items):
                kk_i, st_i = items[i]
                flush_due(i - FBURST, need_st=st_i if kk_i == "OB" else None)
                post(i)
                if kk_i != "OB":
                    pending.append((i + DELAY, i))
                if i + LOOKAHEAD < n_items:
                    kk_p, st_p = items[i + LOOKAHEAD]
                    if kk_p in ("E", "OA"):
                        flush_due(-10**9, need_st=st_p)
                    pre(i + LOOKAHEAD)
            flush_due(10**9)

    orig_tables = bacc.get_activation_tables

    def _lnexp_only(arch):
        return {k: (s if k == "natural_log_exp_and_others" else set())
                for k, s in orig_tables(arch).items()}

    bacc.get_activation_tables = _lnexp_only
    try:
        nc.finalize()
    finally:
        bacc.get_activation_tables = orig_tables
    return nc


_NC_CACHE = {}


def _get_nc(key=None):
    if key is None:
        key = next(iter(_NC_CACHE), (2, 4))
    if key not in _NC_CACHE:
        _NC_CACHE[key] = _build(*key)
    return _NC_CACHE[key]


def kernel(att_scores: np.ndarray, alpha: np.ndarray) -> np.ndarray:
    X = np.asarray(att_scores, dtype=np.float32).reshape(B * H, Q, K)
    al = np.asarray(alpha, dtype=np.float64).reshape(H)
    s_h = al - 1.0

    xh = set(int(h) for h in np.where(s_h >= XHARD_S)[0])
    while (len(xh) * B) % NCORES != 0:
        rest = [h for h in range(H) if h not in xh]
        xh.add(int(max(rest, key=lambda h: s_h[h])))
    hh = set(int(h) for h in np.where(s_h >= HARD_S)[0] if h not in xh)
    while (len(hh) * B) % NCORES != 0:
        rest = [h for h in range(H) if h not in xh and h not in hh]
        hh.add(int(max(rest, key=lambda h: s_h[h])))
    x_blocks = [g for g in range(B * H) if (g % H) in xh]
    h_blocks = [g for g in range(B * H) if (g % H) in hh]
    e_blocks = [g for g in range(B * H)
                if (g % H) not in xh and (g % H) not in hh]
    n_x_b = len(x_blocks) // NCORES
    n_h_b = len(h_blocks) // NCORES
    n_e_b = BLOCKS - n_x_b - n_h_b

    nc = _get_nc((n_x_b * 2, n_h_b * 2))

    assign = []
    for c in range(NCORES):
        assign.append(x_blocks[c * n_x_b:(c + 1) * n_x_b]
                      + h_blocks[c * n_h_b:(c + 1) * n_h_b]
                      + e_blocks[c * n_e_b:(c + 1) * n_e_b])

    lnK = float(np.log(K))
    in_maps = []
    for c in range(NCORES):
        xc = np.ascontiguousarray(
            np.concatenate([X[g] for g in assign[c]], axis=0)
        ).astype(np.float16)
        cvec = np.zeros((10, NC), np.float64)
        for st in range(N_ST):
            h = assign[c][st // (Q // ST_ROWS)] % H
            s = s_h[h]
            a, b, c2 = _fit_coeffs(s)
            cols = slice(st * R, st * R + R)
            cvec[0, cols] = s
            cvec[1, cols] = 1.0 / s
            cvec[2, cols] = 1.0 / s - 1.0
            cvec[3, cols] = 1.0 / s
            cvec[4, cols] = (1.0 / K) ** s / s
            cvec[5, cols] = a
            cvec[6, cols] = b
            cvec[7, cols] = c2
            cvec[8, cols] = -s * lnK
            cvec[9, cols] = (s * UEPS) ** (1.0 / s - 1.0)
        cst = np.tile(cvec.reshape(1, 10 * NC).astype(np.float32), (128, 1))
        in_maps.append({"x": xc, "cst": cst})

    res = run_bass_kernel_spmd(nc, in_maps, core_ids=list(range(NCORES)))
    global LAST_RESULT
    LAST_RESULT = res
    out = np.empty((B * H, Q, K), np.float32)
    for c in range(NCORES):
        yc = np.asarray(res.results[c]["y"]).astype(np.float32)
        yc = yc.reshape(BLOCKS, Q, K)
        for slot, g in enumerate(assign[c]):
            out[g] = yc[slot]
    return out.reshape(B, H, Q, K)


# revision 30
# speedup vs baseline: 1.0216x; 1.0216x over previous
"""Entmax-alpha Bass kernel for Trainium2, 8-core SPMD — v3.

Algorithm (vs v2's AB regula falsi with 5 pow-evals / 10 big ACT passes per
supertile at 606us): fitted initializer + safeguarded Newton + Taylor-
corrected output eval.

  1. init (1 ACT pass): Z = sum exp(4*(x-mx)). The root's temperature
     c* = s*(mx-tau*) is tightly predicted per row by a per-head quadratic
     in lnZ (coefficients fitted offline on synthetic N(0,1) rows,
     interpolated in s from FIT_TABLE). t0 = mx - c/s, clipped to the
     guaranteed bracket [mx-1/s, mx-K^{-s}/s].
  2. Newton evals (2 big ACT passes each: Ln + Exp_{p-1}); 1 for easy
     heads, 2 for hard (s>=0.55), 3 for xhard (s>=0.70):
     u = max(x-t, eps); L = ln(s*u); w1 = exp((p-1)*L) with per-subrow
     accum -> S1; S = s*sum(w1*u) (exact: w1*u = (su)^p / s) via a 2x tt
     mult + fold-sum.  For hard STs the eps-plateau's pollution of S1
     (large when p-1 < 1) is removed exactly: an is_le mask counts clamped
     elements and update() subtracts n*(s*eps)^(p-1); eps = 2^-23 is
     exactly f16-representable so the plateau value is exact.  Newton in
     h=lnS space: dt = h*S/S1 (p*s==1).  f is convex decreasing so Newton
     never overshoots from the f>=1 side; steps are clipped to the static
     bracket.
  3. corrected output eval, split into OA (Ln + Exp_p accum -> w,S +
     Exp_{p-1} -> w1, fold-sum S1) and OB: the *next* Newton step
     dhat = h*S/S1 (bracket-clipped, which also guards S1 blow-ups) is
     applied as a first-order elementwise Taylor update
     wc = w - dhat*p*s*w1 in one scalar_tensor_tensor whose accum gives
     the exact new sum; the final normalize clamps negatives (op1=max).
     Equivalent accuracy to a full extra eval at ~1/3 the ACT cost.

  I/O: host uploads x pre-cast to f16 (halves DMA) and reads f16 y back.
  x streams from HBM per eval (no SBUF residency; DMA has large slack).
  Ln output lives in PSUM (no matmuls -> otherwise idle).  State updates
  flush in bursts so contiguous same-kind supertile runs batch into single
  [128, n*R] tiny ops.  Measured: 545us (vs 606us v2), absmax rel err
  7.2e-3 (harness gate 2e-2).
"""

import os as _os

import numpy as np

import concourse.bacc as bacc
import concourse.mybir as mybir
from concourse.tile import TileContext
from concourse.bass_utils import run_bass_kernel_spmd

B, H, Q, K = 4, 16, 1024, 1024
NCORES = 8
BLOCKS = (B * H) // NCORES      # head-blocks per core (8)
R = 4                           # q-subrows per partition per supertile
ST_ROWS = 128 * R               # rows per supertile (512)
N_ST = BLOCKS * Q // ST_ROWS    # supertiles per core (16)
NC = N_ST * R                   # state columns (64)
KH = K // 2

BETA = 4.0                      # init Exp scale (matches FIT_TABLE fit)
UEPS = float(2.0 ** -23)        # clamp floor for the Ln input (exact f16)
HARD_S = float(_os.environ.get("HARDS", "0.55"))
XHARD_S = float(_os.environ.get("XHARDS", "0.70"))
LOOKAHEAD = int(_os.environ.get("LOOKAHEAD", "3"))
DELAY = int(_os.environ.get("DELAY", "2"))
OBGAP = int(_os.environ.get("OBGAP", "3"))
FBURST = int(_os.environ.get("FBURST", "8"))
ACC_INIT = _os.environ.get("ACCINIT", "0") == "1"   # init Z via ACT accum
ACC_S = _os.environ.get("ACCS", "0") == "1"         # OA S via ACT accum
CPOOL = int(_os.environ.get("CPOOL", "2"))          # clamp subrows on Pool
SPOOL = int(_os.environ.get("SPOOL", "0"))          # fold subrows on Pool
OBPOOL = int(_os.environ.get("OBPOOL", "0"))        # corr subrows on Pool
PSUM_L = _os.environ.get("PSUML", "1") == "1"

# lc ~ a + b*lnZ + c*lnZ^2 fitted on synthetic N(0,1) rows, K=1024, beta=4
FIT_TABLE = [
    (0.0400, -0.1163303, -0.0414107, +0.0076434),
    (0.0800, -0.1912820, -0.0863653, +0.0147054),
    (0.1200, -0.2329487, -0.1340005, +0.0218060),
    (0.1600, -0.2495165, -0.1863370, +0.0304684),
    (0.2000, -0.2572720, -0.2289561, +0.0352949),
    (0.2400, -0.2556715, -0.2690348, +0.0390864),
    (0.2800, -0.2496982, -0.3072143, +0.0430376),
    (0.3200, -0.2448862, -0.3364092, +0.0437538),
    (0.3600, -0.2305074, -0.3786700, +0.0503131),
    (0.4000, -0.2158057, -0.4129058, +0.0530508),
    (0.4400, -0.2088186, -0.4307900, +0.0499715),
    (0.4800, -0.1916382, -0.4675049, +0.0551991),
    (0.5200, -0.1826100, -0.4878948, +0.0542270),
    (0.5600, -0.1674966, -0.5179781, +0.0586533),
    (0.6000, -0.1577472, -0.5331484, +0.0558217),
    (0.6400, -0.1447803, -0.5551327, +0.0563687),
    (0.6800, -0.1283865, -0.5840495, +0.0592415),
    (0.7200, -0.1095254, -0.6217898, +0.0690736),
    (0.7600, -0.1087213, -0.6163985, +0.0585828),
    (0.8000, -0.0942934, -0.6405565, +0.0602640),
    (0.8400, -0.0818819, -0.6659166, +0.0671857),
    (0.8800, -0.0788647, -0.6583176, +0.0556963),
    (0.9200, -0.0652865, -0.6792696, +0.0572530),
    (0.9600, -0.0577361, -0.6821607, +0.0496152),
    (1.0000, -0.0342309, -0.7223812, +0.0618745),
]

AL = mybir.AluOpType
AF = mybir.ActivationFunctionType
F32 = mybir.dt.float32
F16 = mybir.dt.float16

LAST_RESULT = None


def _fit_coeffs(s):
    xs = np.array([r[0] for r in FIT_TABLE])
    return [float(np.interp(s, xs, np.array([r[ci] for r in FIT_TABLE])))
            for ci in range(1, 4)]


def _build(n_x_st, n_hard_st):
    nc = bacc.Bacc(None, target_bir_lowering=False)
    x_in = nc.declare_dram_parameter("x", [BLOCKS * Q, K], F16,
                                     isOutput=False)
    cst_in = nc.declare_dram_parameter("cst", [128, 10 * NC], F32,
                                       isOutput=False)
    y_out = nc.declare_dram_parameter("y", [BLOCKS * Q, K], F16,
                                      isOutput=True)

    with TileContext(nc) as tc:
        with tc.tile_pool(name="state", bufs=1) as stp, \
             tc.tile_pool(name="xt", bufs=4) as xp, \
             tc.tile_pool(name="work", bufs=3) as wp, \
             tc.tile_pool(name="keep", bufs=3) as kp, \
             tc.tile_pool(name="scr", bufs=2) as scp, \
             (tc.tile_pool(name="lps", bufs=1, space="PSUM") if PSUM_L
              else tc.tile_pool(name="lsb", bufs=1)) as lp:
            v = nc.vector
            g = nc.gpsimd

            cst = stp.tile([128, 10 * NC], F32)
            nc.sync.dma_start(cst[:, :], cst_in[:, :])
            sC = cst[:, 0 * NC:1 * NC]     # s
            pC = cst[:, 1 * NC:2 * NC]     # p = 1/s
            pm1C = cst[:, 2 * NC:3 * NC]   # p - 1
            isC = cst[:, 3 * NC:4 * NC]    # 1/s
            kisC = cst[:, 4 * NC:5 * NC]   # K^{-s}/s
            aC = cst[:, 5 * NC:6 * NC]     # init fit a
            bC = cst[:, 6 * NC:7 * NC]     # init fit b
            cC = cst[:, 7 * NC:8 * NC]     # init fit c2
            lminC = cst[:, 8 * NC:9 * NC]  # -s*ln(K)
            epwC = cst[:, 9 * NC:10 * NC]  # (s*UEPS)^(p-1)

            mx = stp.tile([128, NC], F32)
            tS = stp.tile([128, NC], F32)
            tpS = stp.tile([128, NC], F32)
            tnS = stp.tile([128, NC], F32)
            SS = stp.tile([128, NC], F32)   # S accum slots
            S1 = stp.tile([128, NC], F32)   # sum(w1) accum slots
            SxS = stp.tile([128, NC], F32)  # sum(w1*u) accum slots
            cnt = stp.tile([128, NC], F32)  # clamped-element counts
            naS = stp.tile([128, NC], F32)  # -(h*S/S1) for the correction
            t1 = stp.tile([128, NC], F32)
            t2 = stp.tile([128, NC], F32)
            t3 = stp.tile([128, NC], F32)

            # touch ACT so the one Ln/Exp table load overlaps the first DMA
            v.memset(t1[:, 0:1], 1.0)
            nc.scalar.activation(t1[:, 0:1], t1[:, 0:1], AF.Ln)

            def x_dram_ap(handle, st):
                r0 = st * ST_ROWS
                return handle[r0:r0 + ST_ROWS, :].rearrange(
                    "(j p) k -> p j k", p=128)

            def sb3(tile_ap):
                return tile_ap.rearrange("p (j k) -> p j k", k=K)

            # ---------- item stream ----------
            # xhard STs: L,E,E,E,OA,OB; hard: L,E,E,OA,OB; easy: L,E,OA,OB
            n_hc = n_x_st + n_hard_st
            def interleave(pairs):
                # pairs: list of (first, second) two-phase items per st;
                # emit firsts leading seconds by OBGAP
                out = []
                firsts = [p[0] for p in pairs]
                seconds = [p[1] for p in pairs]
                n = len(pairs)
                fi = si = 0
                while si < n:
                    if fi < n and fi - si < OBGAP:
                        out.append(firsts[fi])
                        fi += 1
                    else:
                        out.append(seconds[si])
                        si += 1
                return out

            xhard = list(range(n_x_st))
            hard = list(range(n_x_st, n_hc))
            easy = list(range(n_hc, N_ST))
            items = []
            items += [("L", st) for st in range(N_ST)]
            items += [("E", st) for st in range(N_ST)]
            items += [("E", st) for st in xhard + hard]
            items += [("E", st) for st in xhard]
            items += interleave([(("OA", st), ("OB", st)) for st in easy])
            items += interleave([(("OA", st), ("OB", st))
                                 for st in xhard + hard])
            n_items = len(items)
            ob_of = {}
            for i, (kk, st) in enumerate(items):
                if kk == "OB":
                    ob_of[st] = i

            live = {}

            def clamp_feed(st):
                """DMA x, clamp u = max(x-t, eps)."""
                cc = st * R
                xt = xp.tile([128, R * K], F16, name="xt")
                nc.sync.dma_start(sb3(xt[:, :]), x_dram_ap(x_in, st))
                u16 = wp.tile([128, R * K], F16, name="u16")
                for j in range(R):
                    sl = slice(j * K, (j + 1) * K)
                    eng = g if j < CPOOL else v
                    eng.tensor_scalar(u16[:, sl], xt[:, sl],
                                      tS[:, cc + j:cc + j + 1], UEPS,
                                      op0=AL.subtract, op1=AL.max)
                return (u16,)

            def pre(idx):
                kind, st = items[idx]
                cc = st * R
                if kind == "L":
                    xt = xp.tile([128, R * K], F16, name="xt")
                    nc.sync.dma_start(sb3(xt[:, :]), x_dram_ap(x_in, st))
                    m1 = scp.tile([128, R * KH], F16, name="m1")
                    a3 = xt[:, :].rearrange("p (j two k) -> p j two k",
                                            two=2, k=KH)
                    v.tensor_tensor(
                        m1[:, :].rearrange("p (j k) -> p j k", k=KH),
                        a3[:, :, 0, :], a3[:, :, 1, :], op=AL.max)
                    m2 = scp.tile([128, R * (KH // 2)], F16, name="m2")
                    b3 = m1[:, :].rearrange("p (j two k) -> p j two k",
                                            two=2, k=KH // 2)
                    v.tensor_tensor(
                        m2[:, :].rearrange("p (j k) -> p j k", k=KH // 2),
                        b3[:, :, 0, :], b3[:, :, 1, :], op=AL.max)
                    v.tensor_reduce(
                        mx[:, cc:cc + R],
                        m2[:, :].rearrange("p (j k) -> p j k", k=KH // 2),
                        axis=mybir.AxisListType.X, op=AL.max)
                    d16 = wp.tile([128, R * K], F16, name="u16")
                    for j in range(R):
                        sl = slice(j * K, (j + 1) * K)
                        eng = g if j < CPOOL else v
                        eng.tensor_scalar(d16[:, sl], xt[:, sl],
                                          mx[:, cc + j:cc + j + 1], None,
                                          op0=AL.subtract)
                    live[idx] = (d16,)
                elif kind in ("E", "OA"):
                    live[idx] = clamp_feed(st)

            def fold_sum(src, dstS, cc):
                zf = scp.tile([128, R * KH], F16, name="zf")
                for j in range(R):
                    eng = g if j < SPOOL else v
                    eng.scalar_tensor_tensor(
                        zf[:, j * KH:(j + 1) * KH],
                        src[:, j * K:j * K + KH], 0.0,
                        src[:, j * K + KH:(j + 1) * K],
                        op0=AL.add, op1=AL.add,
                        accum_out=dstS[:, cc + j:cc + j + 1])

            def post(idx):
                kind, st = items[idx]
                cc = st * R
                if kind == "L":
                    (d16,) = live.pop(idx)
                    if ACC_INIT:
                        e16 = scp.tile([128, K], F16, name="e16")
                        for j in range(R):
                            nc.scalar.activation(
                                e16[:, :], d16[:, j * K:(j + 1) * K],
                                AF.Exp, scale=BETA,
                                accum_out=SS[:, cc + j:cc + j + 1])
                    else:
                        e16 = wp.tile([128, R * K], F16, name="w1s")
                        nc.scalar.activation(e16[:, :], d16[:, :], AF.Exp,
                                             scale=BETA)
                        fold_sum(e16, SS, cc)
                    return
                if kind == "OB":
                    w16, w1 = live.pop(idx)
                    wc = wp.tile([128, R * K], F16, name="u16")
                    for j in range(R):
                        sl = slice(j * K, (j + 1) * K)
                        eng = g if j < OBPOOL else v
                        eng.scalar_tensor_tensor(
                            wc[:, sl], w1[:, sl],
                            naS[:, cc + j:cc + j + 1], w16[:, sl],
                            op0=AL.mult, op1=AL.add,
                            accum_out=SS[:, cc + j:cc + j + 1])
                    v.reciprocal(t1[:, cc:cc + R], SS[:, cc:cc + R])
                    dap = x_dram_ap(y_out, st)
                    for j in range(R):
                        sl = slice(j * K, (j + 1) * K)
                        eng = g if j < CPOOL else v
                        eng.tensor_scalar(wc[:, sl], wc[:, sl],
                                          t1[:, cc + j:cc + j + 1], 0.0,
                                          op0=AL.mult, op1=AL.max)
                    nc.sync.dma_start(dap[:, 0:2, :], sb3(wc[:, 0:2 * K]))
                    nc.sync.dma_start(dap[:, 2:4, :],
                                      sb3(wc[:, 2 * K:4 * K])[:, 0:2, :])
                    return
                (u16,) = live.pop(idx)
                Lt = lp.tile([128, R * K], F32, name="L", tag="L")
                nc.scalar.activation(Lt[:, :], u16[:, :], AF.Ln,
                                     scale=sC[:, cc:cc + 1])
                if kind == "E":
                    # w1 = exp((p-1)L) with accum -> S1; S = s*sum(w1*u)
                    w1 = wp.tile([128, R * K], F16, name="w1s")
                    for j in range(R):
                        sl = slice(j * K, (j + 1) * K)
                        nc.scalar.activation(
                            w1[:, sl], Lt[:, sl], AF.Exp,
                            scale=pm1C[:, cc:cc + 1],
                            accum_out=S1[:, cc + j:cc + j + 1])
                    xw = wp.tile([128, R * K], F16, name="xw")
                    v.tensor_tensor(xw[:, :], u16[:, :], w1[:, :],
                                    op=AL.mult)
                    fold_sum(xw, SS, cc)
                else:  # OA: both exps with accum; keep w, w1 for OB
                    w16 = kp.tile([128, R * K], F16, name="w16")
                    for j in range(R):
                        sl = slice(j * K, (j + 1) * K)
                        nc.scalar.activation(
                            w16[:, sl], Lt[:, sl], AF.Exp,
                            scale=pC[:, cc:cc + 1],
                            accum_out=SS[:, cc + j:cc + j + 1])
                    w1 = kp.tile([128, R * K], F16, name="w1k")
                    if ACC_S:
                        for j in range(R):
                            sl = slice(j * K, (j + 1) * K)
                            nc.scalar.activation(
                                w1[:, sl], Lt[:, sl], AF.Exp,
                                scale=pm1C[:, cc:cc + 1],
                                accum_out=S1[:, cc + j:cc + j + 1])
                    else:
                        nc.scalar.activation(w1[:, :], Lt[:, :], AF.Exp,
                                             scale=pm1C[:, cc:cc + 1])
                        fold_sum(w1, S1, cc)
                    live[ob_of[st]] = (w16, w1)
                if st < n_hc:
                    # (s*eps)^(p-1) plateau pollutes S1 when p-1 < 1:
                    # count clamped elements, subtract exactly in update()
                    mk = scp.tile([128, R * K], F16, name="mk")
                    v.tensor_scalar(mk[:, :], u16[:, :], UEPS * 1.5, None,
                                    op0=AL.is_le)
                    fold_sum(mk, cnt, cc)

            def update_run(kind, st0, st1):
                st = st0
                cg = slice(st0 * R, st1 * R + R)
                if kind == "OB":
                    return
                if kind == "L":
                    # t0 = mx - exp(clip(a + lnZ*(b + c*lnZ), lmin, 0))/s
                    nc.scalar.activation(t1[:, cg], SS[:, cg], AF.Ln)
                    v.tensor_tensor(t2[:, cg], cC[:, cg], t1[:, cg],
                                    op=AL.mult)
                    v.tensor_tensor(t2[:, cg], t2[:, cg], bC[:, cg],
                                    op=AL.add)
                    v.tensor_tensor(t2[:, cg], t2[:, cg], t1[:, cg],
                                    op=AL.mult)
                    v.tensor_tensor(t2[:, cg], t2[:, cg], aC[:, cg],
                                    op=AL.add)
                    v.tensor_tensor(t2[:, cg], t2[:, cg], lminC[:, cg],
                                    op=AL.max)
                    v.tensor_scalar_min(t2[:, cg], t2[:, cg], 0.0)
                    nc.scalar.activation(t2[:, cg], t2[:, cg], AF.Exp)
                    v.tensor_tensor(t2[:, cg], t2[:, cg], isC[:, cg],
                                    op=AL.mult)
                    v.tensor_tensor(tS[:, cg], mx[:, cg], t2[:, cg],
                                    op=AL.subtract)
                    v.tensor_tensor(tpS[:, cg], mx[:, cg], isC[:, cg],
                                    op=AL.subtract)
                    v.tensor_tensor(tnS[:, cg], mx[:, cg], kisC[:, cg],
                                    op=AL.subtract)
                    return
                # E / OA: h = ln S; q = h*S/S1  (S1 = sum (su)^(p-1))
                if st < n_hc:
                    # exact removal of the eps-plateau from S1
                    v.tensor_tensor(t2[:, cg], cnt[:, cg], epwC[:, cg],
                                    op=AL.mult)
                    v.tensor_tensor(t2[:, cg], S1[:, cg], t2[:, cg],
                                    op=AL.subtract)
                    v.tensor_scalar_max(t2[:, cg], t2[:, cg], 1e-30)
                else:
                    v.tensor_scalar_max(t2[:, cg], S1[:, cg], 1e-30)
                v.reciprocal(t2[:, cg], t2[:, cg])
                if kind == "E":
                    # SS holds sum(w1*u); S = s*SS.  dt = h*S/S1.
                    v.tensor_tensor(t3[:, cg], SS[:, cg], sC[:, cg],
                                    op=AL.mult)
                    nc.scalar.activation(t1[:, cg], t3[:, cg], AF.Ln)
                    v.tensor_tensor(t1[:, cg], t1[:, cg], SS[:, cg],
                                    op=AL.mult)
                    v.tensor_tensor(t1[:, cg], t1[:, cg], t2[:, cg],
                                    op=AL.mult)
                    v.tensor_tensor(t1[:, cg], t1[:, cg], sC[:, cg],
                                    op=AL.mult)
                    v.tensor_tensor(tS[:, cg], tS[:, cg], t1[:, cg],
                                    op=AL.add)
                    v.tensor_tensor(tS[:, cg], tS[:, cg], tpS[:, cg],
                                    op=AL.max)
                    v.tensor_tensor(tS[:, cg], tS[:, cg], tnS[:, cg],
                                    op=AL.min)
                else:  # OA: SS holds S.  dhat = h*S/S1, bracket-clipped
                    nc.scalar.activation(t1[:, cg], SS[:, cg], AF.Ln)
                    v.tensor_tensor(t1[:, cg], t1[:, cg], SS[:, cg],
                                    op=AL.mult)
                    v.tensor_tensor(t1[:, cg], t1[:, cg], t2[:, cg],
                                    op=AL.mult)
                    # clip dhat into [tp - t, tn - t] (guards S1 blowups)
                    v.tensor_tensor(t2[:, cg], tpS[:, cg], tS[:, cg],
                                    op=AL.subtract)
                    v.tensor_tensor(t1[:, cg], t1[:, cg], t2[:, cg],
                                    op=AL.max)
                    v.tensor_tensor(t2[:, cg], tnS[:, cg], tS[:, cg],
                                    op=AL.subtract)
                    v.tensor_tensor(t1[:, cg], t1[:, cg], t2[:, cg],
                                    op=AL.min)
                    v.tensor_scalar_mul(naS[:, cg], t1[:, cg], -1.0)

            # ---------- pipelined emission ----------
            # updates flush in bursts so contiguous same-kind runs batch
            # into single [128, n*R] ops (saves tiny-op overhead); any
            # pre/post that depends on an st's pending update force-flushes
            # it first.
            def flush(entries):
                runs = []
                for (due, idx) in entries:
                    kk, st = items[idx]
                    if (runs and runs[-1][0] == kk
                            and st == runs[-1][2] + 1 and kk != "OB"
                            and (st < n_hc) == (runs[-1][1] < n_hc)):
                        runs[-1][2] = st
                    else:
                        runs.append([kk, st, st])
                for kk, st0, st1 in runs:
                    update_run(kk, st0, st1)

            pending = []

            def flush_due(i, need_st=None):
                take = [e for e in pending
                        if e[0] <= i
                        or (need_st is not None and items[e[1]][1] == need_st)]
                if not take:
                    return
                for e in take:
                    pending.remove(e)
                flush(sorted(take, key=lambda e: e[1]))

            for i in range(min(LOOKAHEAD, n_items)):
                pre(i)
            for i in range(n_items):
                kk_i, st_i = items[i]
                flush_due(i - FBURST, need_st=st_i if kk_i == "OB" else None)
                post(i)
                if kk_i != "OB":
                    pending.append((i + DELAY, i))
                if i + LOOKAHEAD < n_items:
                    kk_p, st_p = items[i + LOOKAHEAD]
                    if kk_p in ("E", "OA"):
                        flush_due(-10**9, need_st=st_p)
                    pre(i + LOOKAHEAD)
            flush_due(10**9)

    orig_tables = bacc.get_activation_tables

    def _lnexp_only(arch):
        return {k: (s if k == "natural_log_exp_and_others" else set())
                for k, s in orig_tables(arch).items()}

    bacc.get_activation_tables = _lnexp_only
    try:
        nc.finalize()
    finally:
        bacc.get_activation_tables = orig_tables
    return nc


_NC_CACHE = {}


def _get_nc(key=None):
    if key is None:
        key = next(iter(_NC_CACHE), (2, 4))
    if key not in _NC_CACHE:
        _NC_CACHE[key] = _build(*key)
    return _NC_CACHE[key]


def kernel(att_scores: np.ndarray, alpha: np.ndarray) -> np.ndarray:
    X = np.asarray(att_scores, dtype=np.float32).reshape(B * H, Q, K)
    al = np.asarray(alpha, dtype=np.float64).reshape(H)
    s_h = al - 1.0

    xh = set(int(h) for h in np.where(s_h >= XHARD_S)[0])
    while (len(xh) * B) % NCORES != 0:
        rest = [h for h in range(H) if h not in xh]
        xh.add(int(max(rest, key=lambda h: s_h[h])))
    hh = set(int(h) for h in np.where(s_h >= HARD_S)[0] if h not in xh)
    while (len(hh) * B) % NCORES != 0:
        rest = [h for h in range(H) if h not in xh and h not in hh]
        hh.add(int(max(rest, key=lambda h: s_h[h])))
    x_blocks = [g for g in range(B * H) if (g % H) in xh]
    h_blocks = [g for g in range(B * H) if (g % H) in hh]
    e_blocks = [g for g in range(B * H)
                if (g % H) not in xh and (g % H) not in hh]
    n_x_b = len(x_blocks) // NCORES
    n_h_b = len(h_blocks) // NCORES
    n_e_b = BLOCKS - n_x_b - n_h_b

    nc = _get_nc((n_x_b * 2, n_h_b * 2))

    assign = []
    for c in range(NCORES):
        assign.append(x_blocks[c * n_x_b:(c + 1) * n_x_b]
                      + h_blocks[c * n_h_b:(c + 1) * n_h_b]
                      + e_blocks[c * n_e_b:(c + 1) * n_e_b])

    lnK = float(np.log(K))
    in_maps = []
    for c in range(NCORES):
        xc = np.ascontiguousarray(
            np.concatenate([X[g] for g in assign[c]], axis=0)
        ).astype(np.float16)
        cvec = np.zeros((10, NC), np.float64)
        for st in range(N_ST):
            h = assign[c][st // (Q // ST_ROWS)] % H
            s = s_h[h]
            a, b, c2 = _fit_coeffs(s)
            cols = slice(st * R, st * R + R)
            cvec[0, cols] = s
            cvec[1, cols] = 1.0 / s
            cvec[2, cols] = 1.0 / s - 1.0
            cvec[3, cols] = 1.0 / s
            cvec[4, cols] = (1.0 / K) ** s / s
            cvec[5, cols] = a
            cvec[6, cols] = b
            cvec[7, cols] = c2
            cvec[8, cols] = -s * lnK
            cvec[9, cols] = (s * UEPS) ** (1.0 / s - 1.0)
        cst = np.tile(cvec.reshape(1, 10 * NC).astype(np.float32), (128, 1))
        in_maps.append({"x": xc, "cst": cst})

    res = run_bass_kernel_spmd(nc, in_maps, core_ids=list(range(NCORES)))
    global LAST_RESULT
    LAST_RESULT = res
    out = np.empty((B * H, Q, K), np.float32)
    for c in range(NCORES):
        yc = np.asarray(res.results[c]["y"]).astype(np.float32)
        yc = yc.reshape(BLOCKS, Q, K)
        for slot, g in enumerate(assign[c]):
            out[g] = yc[slot]
    return out.reshape(B, H, Q, K)


# revision 31
# speedup vs baseline: 1.0354x; 1.0135x over previous
"""Entmax-alpha Bass kernel for Trainium2, 8-core SPMD — v3.

Algorithm (vs v2's AB regula falsi with 5 pow-evals / 10 big ACT passes per
supertile at 606us): fitted initializer + safeguarded Newton + Taylor-
corrected output eval.

  1. init (1 ACT pass): Z = sum exp(4*(x-mx)). The root's temperature
     c* = s*(mx-tau*) is tightly predicted per row by a per-head quadratic
     in lnZ (coefficients fitted offline on synthetic N(0,1) rows,
     interpolated in s from FIT_TABLE). t0 = mx - c/s, clipped to the
     guaranteed bracket [mx-1/s, mx-K^{-s}/s].
  2. Newton evals (2 big ACT passes each: Ln + Exp_{p-1}); 1 for easy
     heads, 2 for hard (s>=0.55), 3 for xhard (s>=0.70):
     u = max(x-t, eps); L = ln(s*u); w1 = exp((p-1)*L) with per-subrow
     accum -> S1; S = s*sum(w1*u) (exact: w1*u = (su)^p / s) via a 2x tt
     mult + fold-sum.  For hard STs the eps-plateau's pollution of S1
     (large when p-1 < 1) is removed exactly: an is_le mask counts clamped
     elements and update() subtracts n*(s*eps)^(p-1); eps = 2^-23 is
     exactly f16-representable so the plateau value is exact.  Newton in
     h=lnS space: dt = h*S/S1 (p*s==1).  f is convex decreasing so Newton
     never overshoots from the f>=1 side; steps are clipped to the static
     bracket.
  3. corrected output eval, split into OA (Ln + Exp_p accum -> w,S +
     Exp_{p-1} -> w1, fold-sum S1) and OB: the *next* Newton step
     dhat = h*S/S1 (bracket-clipped, which also guards S1 blow-ups) is
     applied as a first-order elementwise Taylor update
     wc = w - dhat*p*s*w1 in one scalar_tensor_tensor whose accum gives
     the exact new sum; the final normalize clamps negatives (op1=max).
     Equivalent accuracy to a full extra eval at ~1/3 the ACT cost.

  I/O: host uploads x pre-cast to f16 (halves DMA) and reads f16 y back.
  x streams from HBM per eval (no SBUF residency; DMA has large slack).
  Ln output lives in PSUM (no matmuls -> otherwise idle).  State updates
  flush in bursts so contiguous same-kind supertile runs batch into single
  [128, n*R] tiny ops.  Measured: 545us (vs 606us v2), absmax rel err
  7.2e-3 (harness gate 2e-2).
"""

import os as _os

import numpy as np

import concourse.bacc as bacc
import concourse.mybir as mybir
from concourse.tile import TileContext
from concourse.bass_utils import run_bass_kernel_spmd

B, H, Q, K = 4, 16, 1024, 1024
NCORES = 8
BLOCKS = (B * H) // NCORES      # head-blocks per core (8)
R = 4                           # q-subrows per partition per supertile
ST_ROWS = 128 * R               # rows per supertile (512)
N_ST = BLOCKS * Q // ST_ROWS    # supertiles per core (16)
NC = N_ST * R                   # state columns (64)
KH = K // 2

BETA = 4.0                      # init Exp scale (matches FIT_TABLE fit)
UEPS = float(2.0 ** -23)        # clamp floor for the Ln input (exact f16)
HARD_S = float(_os.environ.get("HARDS", "0.55"))
XHARD_S = float(_os.environ.get("XHARDS", "0.70"))
LOOKAHEAD = int(_os.environ.get("LOOKAHEAD", "3"))
DELAY = int(_os.environ.get("DELAY", "2"))
OBGAP = int(_os.environ.get("OBGAP", "3"))
FBURST = int(_os.environ.get("FBURST", "8"))
ACC_INIT = _os.environ.get("ACCINIT", "1") == "1"   # init Z via ACT accum
ACC_S = _os.environ.get("ACCS", "0") == "1"         # OA S via ACT accum
CPOOL = int(_os.environ.get("CPOOL", "2"))          # clamp subrows on Pool
SPOOL = int(_os.environ.get("SPOOL", "0"))          # fold subrows on Pool
OBPOOL = int(_os.environ.get("OBPOOL", "0"))        # corr subrows on Pool
PSUM_L = _os.environ.get("PSUML", "1") == "1"

# lc ~ a + b*lnZ + c*lnZ^2 fitted on synthetic N(0,1) rows, K=1024, beta=4
FIT_TABLE = [
    (0.0400, -0.1163303, -0.0414107, +0.0076434),
    (0.0800, -0.1912820, -0.0863653, +0.0147054),
    (0.1200, -0.2329487, -0.1340005, +0.0218060),
    (0.1600, -0.2495165, -0.1863370, +0.0304684),
    (0.2000, -0.2572720, -0.2289561, +0.0352949),
    (0.2400, -0.2556715, -0.2690348, +0.0390864),
    (0.2800, -0.2496982, -0.3072143, +0.0430376),
    (0.3200, -0.2448862, -0.3364092, +0.0437538),
    (0.3600, -0.2305074, -0.3786700, +0.0503131),
    (0.4000, -0.2158057, -0.4129058, +0.0530508),
    (0.4400, -0.2088186, -0.4307900, +0.0499715),
    (0.4800, -0.1916382, -0.4675049, +0.0551991),
    (0.5200, -0.1826100, -0.4878948, +0.0542270),
    (0.5600, -0.1674966, -0.5179781, +0.0586533),
    (0.6000, -0.1577472, -0.5331484, +0.0558217),
    (0.6400, -0.1447803, -0.5551327, +0.0563687),
    (0.6800, -0.1283865, -0.5840495, +0.0592415),
    (0.7200, -0.1095254, -0.6217898, +0.0690736),
    (0.7600, -0.1087213, -0.6163985, +0.0585828),
    (0.8000, -0.0942934, -0.6405565, +0.0602640),
    (0.8400, -0.0818819, -0.6659166, +0.0671857),
    (0.8800, -0.0788647, -0.6583176, +0.0556963),
    (0.9200, -0.0652865, -0.6792696, +0.0572530),
    (0.9600, -0.0577361, -0.6821607, +0.0496152),
    (1.0000, -0.0342309, -0.7223812, +0.0618745),
]

AL = mybir.AluOpType
AF = mybir.ActivationFunctionType
F32 = mybir.dt.float32
F16 = mybir.dt.float16

LAST_RESULT = None


def _fit_coeffs(s):
    xs = np.array([r[0] for r in FIT_TABLE])
    return [float(np.interp(s, xs, np.array([r[ci] for r in FIT_TABLE])))
            for ci in range(1, 4)]


def _build(n_x_st, n_hard_st):
    nc = bacc.Bacc(None, target_bir_lowering=False)
    x_in = nc.declare_dram_parameter("x", [BLOCKS * Q, K], F16,
                                     isOutput=False)
    cst_in = nc.declare_dram_parameter("cst", [128, 10 * NC], F32,
                                       isOutput=False)
    y_out = nc.declare_dram_parameter("y", [BLOCKS * Q, K], F16,
                                      isOutput=True)

    with TileContext(nc) as tc:
        with tc.tile_pool(name="state", bufs=1) as stp, \
             tc.tile_pool(name="xt", bufs=4) as xp, \
             tc.tile_pool(name="work", bufs=3) as wp, \
             tc.tile_pool(name="keep", bufs=3) as kp, \
             tc.tile_pool(name="scr", bufs=2) as scp, \
             (tc.tile_pool(name="lps", bufs=1, space="PSUM") if PSUM_L
              else tc.tile_pool(name="lsb", bufs=1)) as lp:
            v = nc.vector
            g = nc.gpsimd

            cst = stp.tile([128, 10 * NC], F32)
            nc.sync.dma_start(cst[:, :], cst_in[:, :])
            sC = cst[:, 0 * NC:1 * NC]     # s
            pC = cst[:, 1 * NC:2 * NC]     # p = 1/s
            pm1C = cst[:, 2 * NC:3 * NC]   # p - 1
            isC = cst[:, 3 * NC:4 * NC]    # 1/s
            kisC = cst[:, 4 * NC:5 * NC]   # K^{-s}/s
            aC = cst[:, 5 * NC:6 * NC]     # init fit a
            bC = cst[:, 6 * NC:7 * NC]     # init fit b
            cC = cst[:, 7 * NC:8 * NC]     # init fit c2
            lminC = cst[:, 8 * NC:9 * NC]  # -s*ln(K)
            epwC = cst[:, 9 * NC:10 * NC]  # (s*UEPS)^(p-1)

            mx = stp.tile([128, NC], F32)
            tS = stp.tile([128, NC], F32)
            tpS = stp.tile([128, NC], F32)
            tnS = stp.tile([128, NC], F32)
            SS = stp.tile([128, NC], F32)   # S accum slots
            S1 = stp.tile([128, NC], F32)   # sum(w1) accum slots
            SxS = stp.tile([128, NC], F32)  # sum(w1*u) accum slots
            cnt = stp.tile([128, NC], F32)  # clamped-element counts
            naS = stp.tile([128, NC], F32)  # -(h*S/S1) for the correction
            t1 = stp.tile([128, NC], F32)
            t2 = stp.tile([128, NC], F32)
            t3 = stp.tile([128, NC], F32)

            # touch ACT so the one Ln/Exp table load overlaps the first DMA
            v.memset(t1[:, 0:1], 1.0)
            nc.scalar.activation(t1[:, 0:1], t1[:, 0:1], AF.Ln)

            def x_dram_ap(handle, st):
                r0 = st * ST_ROWS
                return handle[r0:r0 + ST_ROWS, :].rearrange(
                    "(j p) k -> p j k", p=128)

            def sb3(tile_ap):
                return tile_ap.rearrange("p (j k) -> p j k", k=K)

            # ---------- item stream ----------
            # xhard STs: L,E,E,E,OA,OB; hard: L,E,E,OA,OB; easy: L,E,OA,OB
            n_hc = n_x_st + n_hard_st
            def interleave(pairs):
                # pairs: list of (first, second) two-phase items per st;
                # emit firsts leading seconds by OBGAP
                out = []
                firsts = [p[0] for p in pairs]
                seconds = [p[1] for p in pairs]
                n = len(pairs)
                fi = si = 0
                while si < n:
                    if fi < n and fi - si < OBGAP:
                        out.append(firsts[fi])
                        fi += 1
                    else:
                        out.append(seconds[si])
                        si += 1
                return out

            xhard = list(range(n_x_st))
            hard = list(range(n_x_st, n_hc))
            easy = list(range(n_hc, N_ST))
            items = []
            items += [("L", st) for st in range(N_ST)]
            items += [("E", st) for st in range(N_ST)]
            items += [("E", st) for st in xhard + hard]
            items += [("E", st) for st in xhard]
            items += interleave([(("OA", st), ("OB", st))
                                 for st in hard + xhard])
            items += interleave([(("OA", st), ("OB", st)) for st in easy])
            n_items = len(items)
            ob_of = {}
            for i, (kk, st) in enumerate(items):
                if kk == "OB":
                    ob_of[st] = i

            live = {}

            def clamp_feed(st):
                """DMA x, clamp u = max(x-t, eps)."""
                cc = st * R
                xt = xp.tile([128, R * K], F16, name="xt")
                nc.sync.dma_start(sb3(xt[:, :]), x_dram_ap(x_in, st))
                u16 = wp.tile([128, R * K], F16, name="u16")
                for j in range(R):
                    sl = slice(j * K, (j + 1) * K)
                    eng = g if j < CPOOL else v
                    eng.tensor_scalar(u16[:, sl], xt[:, sl],
                                      tS[:, cc + j:cc + j + 1], UEPS,
                                      op0=AL.subtract, op1=AL.max)
                return (u16,)

            def pre(idx):
                kind, st = items[idx]
                cc = st * R
                if kind == "L":
                    xt = xp.tile([128, R * K], F16, name="xt")
                    nc.sync.dma_start(sb3(xt[:, :]), x_dram_ap(x_in, st))
                    m1 = scp.tile([128, R * KH], F16, name="m1")
                    a3 = xt[:, :].rearrange("p (j two k) -> p j two k",
                                            two=2, k=KH)
                    v.tensor_tensor(
                        m1[:, :].rearrange("p (j k) -> p j k", k=KH),
                        a3[:, :, 0, :], a3[:, :, 1, :], op=AL.max)
                    m2 = scp.tile([128, R * (KH // 2)], F16, name="m2")
                    b3 = m1[:, :].rearrange("p (j two k) -> p j two k",
                                            two=2, k=KH // 2)
                    v.tensor_tensor(
                        m2[:, :].rearrange("p (j k) -> p j k", k=KH // 2),
                        b3[:, :, 0, :], b3[:, :, 1, :], op=AL.max)
                    v.tensor_reduce(
                        mx[:, cc:cc + R],
                        m2[:, :].rearrange("p (j k) -> p j k", k=KH // 2),
                        axis=mybir.AxisListType.X, op=AL.max)
                    d16 = wp.tile([128, R * K], F16, name="u16")
                    for j in range(R):
                        sl = slice(j * K, (j + 1) * K)
                        eng = g if j < CPOOL else v
                        eng.tensor_scalar(d16[:, sl], xt[:, sl],
                                          mx[:, cc + j:cc + j + 1], None,
                                          op0=AL.subtract)
                    live[idx] = (d16,)
                elif kind in ("E", "OA"):
                    live[idx] = clamp_feed(st)

            def fold_sum(src, dstS, cc):
                zf = scp.tile([128, R * KH], F16, name="zf")
                for j in range(R):
                    eng = g if j < SPOOL else v
                    eng.scalar_tensor_tensor(
                        zf[:, j * KH:(j + 1) * KH],
                        src[:, j * K:j * K + KH], 0.0,
                        src[:, j * K + KH:(j + 1) * K],
                        op0=AL.add, op1=AL.add,
                        accum_out=dstS[:, cc + j:cc + j + 1])

            def post(idx):
                kind, st = items[idx]
                cc = st * R
                if kind == "L":
                    (d16,) = live.pop(idx)
                    if ACC_INIT:
                        e16 = scp.tile([128, K], F16, name="e16")
                        for j in range(R):
                            nc.scalar.activation(
                                e16[:, :], d16[:, j * K:(j + 1) * K],
                                AF.Exp, scale=BETA,
                                accum_out=SS[:, cc + j:cc + j + 1])
                    else:
                        e16 = wp.tile([128, R * K], F16, name="w1s")
                        nc.scalar.activation(e16[:, :], d16[:, :], AF.Exp,
                                             scale=BETA)
                        fold_sum(e16, SS, cc)
                    return
                if kind == "OB":
                    w16, w1 = live.pop(idx)
                    wc = wp.tile([128, R * K], F16, name="u16")
                    for j in range(R):
                        sl = slice(j * K, (j + 1) * K)
                        eng = g if j < OBPOOL else v
                        eng.scalar_tensor_tensor(
                            wc[:, sl], w1[:, sl],
                            naS[:, cc + j:cc + j + 1], w16[:, sl],
                            op0=AL.mult, op1=AL.add,
                            accum_out=SS[:, cc + j:cc + j + 1])
                    v.reciprocal(t1[:, cc:cc + R], SS[:, cc:cc + R])
                    dap = x_dram_ap(y_out, st)
                    for j in range(R):
                        sl = slice(j * K, (j + 1) * K)
                        eng = g if j < CPOOL else v
                        eng.tensor_scalar(wc[:, sl], wc[:, sl],
                                          t1[:, cc + j:cc + j + 1], 0.0,
                                          op0=AL.mult, op1=AL.max)
                    nc.sync.dma_start(dap[:, 0:2, :], sb3(wc[:, 0:2 * K]))
                    nc.sync.dma_start(dap[:, 2:4, :],
                                      sb3(wc[:, 2 * K:4 * K])[:, 0:2, :])
                    return
                (u16,) = live.pop(idx)
                Lt = lp.tile([128, R * K], F32, name="L", tag="L")
                nc.scalar.activation(Lt[:, :], u16[:, :], AF.Ln,
                                     scale=sC[:, cc:cc + 1])
                if kind == "E":
                    # w1 = exp((p-1)L) with accum -> S1; S = s*sum(w1*u)
                    w1 = wp.tile([128, R * K], F16, name="w1s")
                    for j in range(R):
                        sl = slice(j * K, (j + 1) * K)
                        nc.scalar.activation(
                            w1[:, sl], Lt[:, sl], AF.Exp,
                            scale=pm1C[:, cc:cc + 1],
                            accum_out=S1[:, cc + j:cc + j + 1])
                    xw = wp.tile([128, R * K], F16, name="xw")
                    v.tensor_tensor(xw[:, :], u16[:, :], w1[:, :],
                                    op=AL.mult)
                    fold_sum(xw, SS, cc)
                else:  # OA: both exps with accum; keep w, w1 for OB
                    w16 = kp.tile([128, R * K], F16, name="w16")
                    for j in range(R):
                        sl = slice(j * K, (j + 1) * K)
                        nc.scalar.activation(
                            w16[:, sl], Lt[:, sl], AF.Exp,
                            scale=pC[:, cc:cc + 1],
                            accum_out=SS[:, cc + j:cc + j + 1])
                    w1 = kp.tile([128, R * K], F16, name="w1k")
                    if ACC_S:
                        for j in range(R):
                            sl = slice(j * K, (j + 1) * K)
                            nc.scalar.activation(
                                w1[:, sl], Lt[:, sl], AF.Exp,
                                scale=pm1C[:, cc:cc + 1],
                                accum_out=S1[:, cc + j:cc + j + 1])
                    else:
                        nc.scalar.activation(w1[:, :], Lt[:, :], AF.Exp,
                                             scale=pm1C[:, cc:cc + 1])
                        fold_sum(w1, S1, cc)
                    live[ob_of[st]] = (w16, w1)
                if st < n_hc:
                    # (s*eps)^(p-1) plateau pollutes S1 when p-1 < 1:
                    # count clamped elements, subtract exactly in update()
                    mk = scp.tile([128, R * K], F16, name="mk")
                    v.tensor_scalar(mk[:, :], u16[:, :], UEPS * 1.5, None,
                                    op0=AL.is_le)
                    fold_sum(mk, cnt, cc)

            def update_run(kind, st0, st1):
                st = st0
                cg = slice(st0 * R, st1 * R + R)
                if kind == "OB":
                    return
                if kind == "L":
                    # t0 = mx - exp(clip(a + lnZ*(b + c*lnZ), lmin, 0))/s
                    nc.scalar.activation(t1[:, cg], SS[:, cg], AF.Ln)
                    v.tensor_tensor(t2[:, cg], cC[:, cg], t1[:, cg],
                                    op=AL.mult)
                    v.tensor_tensor(t2[:, cg], t2[:, cg], bC[:, cg],
                                    op=AL.add)
                    v.tensor_tensor(t2[:, cg], t2[:, cg], t1[:, cg],
                                    op=AL.mult)
                    v.tensor_tensor(t2[:, cg], t2[:, cg], aC[:, cg],
                                    op=AL.add)
                    v.tensor_tensor(t2[:, cg], t2[:, cg], lminC[:, cg],
                                    op=AL.max)
                    v.tensor_scalar_min(t2[:, cg], t2[:, cg], 0.0)
                    nc.scalar.activation(t2[:, cg], t2[:, cg], AF.Exp)
                    v.tensor_tensor(t2[:, cg], t2[:, cg], isC[:, cg],
                                    op=AL.mult)
                    v.tensor_tensor(tS[:, cg], mx[:, cg], t2[:, cg],
                                    op=AL.subtract)
                    v.tensor_tensor(tpS[:, cg], mx[:, cg], isC[:, cg],
                                    op=AL.subtract)
                    v.tensor_tensor(tnS[:, cg], mx[:, cg], kisC[:, cg],
                                    op=AL.subtract)
                    return
                # E / OA: h = ln S; q = h*S/S1  (S1 = sum (su)^(p-1))
                if st < n_hc:
                    # exact removal of the eps-plateau from S1
                    v.tensor_tensor(t2[:, cg], cnt[:, cg], epwC[:, cg],
                                    op=AL.mult)
                    v.tensor_tensor(t2[:, cg], S1[:, cg], t2[:, cg],
                                    op=AL.subtract)
                    v.tensor_scalar_max(t2[:, cg], t2[:, cg], 1e-30)
                else:
                    v.tensor_scalar_max(t2[:, cg], S1[:, cg], 1e-30)
                v.reciprocal(t2[:, cg], t2[:, cg])
                if kind == "E":
                    # SS holds sum(w1*u); S = s*SS.  dt = h*S/S1.
                    v.tensor_tensor(t3[:, cg], SS[:, cg], sC[:, cg],
                                    op=AL.mult)
                    nc.scalar.activation(t1[:, cg], t3[:, cg], AF.Ln)
                    v.tensor_tensor(t1[:, cg], t1[:, cg], SS[:, cg],
                                    op=AL.mult)
                    v.tensor_tensor(t1[:, cg], t1[:, cg], t2[:, cg],
                                    op=AL.mult)
                    v.tensor_tensor(t1[:, cg], t1[:, cg], sC[:, cg],
                                    op=AL.mult)
                    v.tensor_tensor(tS[:, cg], tS[:, cg], t1[:, cg],
                                    op=AL.add)
                    v.tensor_tensor(tS[:, cg], tS[:, cg], tpS[:, cg],
                                    op=AL.max)
                    v.tensor_tensor(tS[:, cg], tS[:, cg], tnS[:, cg],
                                    op=AL.min)
                else:  # OA: SS holds S.  dhat = h*S/S1, bracket-clipped
                    nc.scalar.activation(t1[:, cg], SS[:, cg], AF.Ln)
                    v.tensor_tensor(t1[:, cg], t1[:, cg], SS[:, cg],
                                    op=AL.mult)
                    v.tensor_tensor(t1[:, cg], t1[:, cg], t2[:, cg],
                                    op=AL.mult)
                    # clip dhat into [tp - t, tn - t] (guards S1 blowups)
                    v.tensor_tensor(t2[:, cg], tpS[:, cg], tS[:, cg],
                                    op=AL.subtract)
                    v.tensor_tensor(t1[:, cg], t1[:, cg], t2[:, cg],
                                    op=AL.max)
                    v.tensor_tensor(t2[:, cg], tnS[:, cg], tS[:, cg],
                                    op=AL.subtract)
                    v.tensor_tensor(t1[:, cg], t1[:, cg], t2[:, cg],
                                    op=AL.min)
                    v.tensor_scalar_mul(naS[:, cg], t1[:, cg], -1.0)

            # ---------- pipelined emission ----------
            # updates flush in bursts so contiguous same-kind runs batch
            # into single [128, n*R] ops (saves tiny-op overhead); any
            # pre/post that depends on an st's pending update force-flushes
            # it first.
            def flush(entries):
                runs = []
                for (due, idx) in entries:
                    kk, st = items[idx]
                    if (runs and runs[-1][0] == kk
                            and st == runs[-1][2] + 1 and kk != "OB"
                            and (st < n_hc) == (runs[-1][1] < n_hc)):
                        runs[-1][2] = st
                    else:
                        runs.append([kk, st, st])
                for kk, st0, st1 in runs:
                    update_run(kk, st0, st1)

            pending = []

            def flush_due(i, need_st=None):
                take = [e for e in pending
                        if e[0] <= i
                        or (need_st is not None and items[e[1]][1] == need_st)]
                if not take:
                    return
                for e in take:
                    pending.remove(e)
                flush(sorted(take, key=lambda e: e[1]))

            for i in range(min(LOOKAHEAD, n_items)):
                pre(i)
            for i in range(n_items):
                kk_i, st_i = items[i]
                flush_due(i - FBURST, need_st=st_i if kk_i == "OB" else None)
                post(i)
                if kk_i != "OB":
                    pending.append((i + DELAY, i))
                if i + LOOKAHEAD < n_items:
                    kk_p, st_p = items[i + LOOKAHEAD]
                    if kk_p in ("E", "OA"):
                        flush_due(-10**9, need_st=st_p)
                    pre(i + LOOKAHEAD)
            flush_due(10**9)

    orig_tables = bacc.get_activation_tables

    def _lnexp_only(arch):
        return {k: (s if k == "natural_log_exp_and_others" else set())
                for k, s in orig_tables(arch).items()}

    bacc.get_activation_tables = _lnexp_only
    try:
        nc.finalize()
    finally:
        bacc.get_activation_tables = orig_tables
    return nc


_NC_CACHE = {}


def _get_nc(key=None):
    if key is None:
        key = next(iter(_NC_CACHE), (2, 4))
    if key not in _NC_CACHE:
        _NC_CACHE[key] = _build(*key)
    return _NC_CACHE[key]


def kernel(att_scores: np.ndarray, alpha: np.ndarray) -> np.ndarray:
    X = np.asarray(att_scores, dtype=np.float32).reshape(B * H, Q, K)
    al = np.asarray(alpha, dtype=np.float64).reshape(H)
    s_h = al - 1.0

    xh = set(int(h) for h in np.where(s_h >= XHARD_S)[0])
    while (len(xh) * B) % NCORES != 0:
        rest = [h for h in range(H) if h not in xh]
        xh.add(int(max(rest, key=lambda h: s_h[h])))
    hh = set(int(h) for h in np.where(s_h >= HARD_S)[0] if h not in xh)
    while (len(hh) * B) % NCORES != 0:
        rest = [h for h in range(H) if h not in xh and h not in hh]
        hh.add(int(max(rest, key=lambda h: s_h[h])))
    x_blocks = [g for g in range(B * H) if (g % H) in xh]
    h_blocks = [g for g in range(B * H) if (g % H) in hh]
    e_blocks = [g for g in range(B * H)
                if (g % H) not in xh and (g % H) not in hh]
    n_x_b = len(x_blocks) // NCORES
    n_h_b = len(h_blocks) // NCORES
    n_e_b = BLOCKS - n_x_b - n_h_b

    nc = _get_nc((n_x_b * 2, n_h_b * 2))

    assign = []
    for c in range(NCORES):
        assign.append(x_blocks[c * n_x_b:(c + 1) * n_x_b]
                      + h_blocks[c * n_h_b:(c + 1) * n_h_b]
                      + e_blocks[c * n_e_b:(c + 1) * n_e_b])

    lnK = float(np.log(K))
    in_maps = []
    for c in range(NCORES):
        xc = np.ascontiguousarray(
            np.concatenate([X[g] for g in assign[c]], axis=0)
        ).astype(np.float16)
        cvec = np.zeros((10, NC), np.float64)
        for st in range(N_ST):
            h = assign[c][st // (Q // ST_ROWS)] % H
            s = s_h[h]
            a, b, c2 = _fit_coeffs(s)
            cols = slice(st * R, st * R + R)
            cvec[0, cols] = s
            cvec[1, cols] = 1.0 / s
            cvec[2, cols] = 1.0 / s - 1.0
            cvec[3, cols] = 1.0 / s
            cvec[4, cols] = (1.0 / K) ** s / s
            cvec[5, cols] = a
            cvec[6, cols] = b
            cvec[7, cols] = c2
            cvec[8, cols] = -s * lnK
            cvec[9, cols] = (s * UEPS) ** (1.0 / s - 1.0)
        cst = np.tile(cvec.reshape(1, 10 * NC).astype(np.float32), (128, 1))
        in_maps.append({"x": xc, "cst": cst})

    res = run_bass_kernel_spmd(nc, in_maps, core_ids=list(range(NCORES)))
    global LAST_RESULT
    LAST_RESULT = res
    out = np.empty((B * H, Q, K), np.float32)
    for c in range(NCORES):
        yc = np.asarray(res.results[c]["y"]).astype(np.float32)
        yc = yc.reshape(BLOCKS, Q, K)
        for slot, g in enumerate(assign[c]):
            out[g] = yc[slot]
    return out.reshape(B, H, Q, K)
